# revision 1
# baseline (speedup 1.0000x reference)
"""Block-causal self-attention on 8 Trainium2 NeuronCores.

Sharding: data-parallel over batch (B=8 -> one batch element per core).
Weights replicated. No collectives.

Per-core Bass program (fp32 storage, fp32r matmuls ~ full PE rate at N>=256):
  - inputs arrive pre-transposed on host: xT=[C,T], w*T=[C,C] (c-major)
  - qT,kT = W @ xT + b   (feature-on-partition layout [C,T])
  - v     = x @ WvT + bv (natural [T,C]) stored per-head with two ones columns
  - per (head, 256-query pair), transposed-scores flash attention:
      scores^T[key, query] = kT_tile.T @ qT  (K=d=64)
      + rank-17 matmul adds the block-causal fine mask as a 0/-8192 bias
        (16 frame-indicator rows x frame-step rows + constant row)
      ACT exp (scale=1/sqrt(d) fused; general mask path adds per-key bias)
      AV: psY[0:66] += v_aug.T @ p  (ones cols give softmax denominator rows)
    only lower-triangular key tiles are computed; diagonal processed first
  - normalize: r = 1/l (DVE), gpsimd partition_broadcast, DVE multiply -> yT
  - out = yT proj + bp (rank-1 bias matmul), ACT copy, DMA per 128-row tile
"""

import contextlib
import math

import numpy as np

import concourse.bass as bass
import concourse.mybir as mybir
import concourse.tile as tile
from concourse import bacc
from concourse.bass_utils import run_bass_kernel_spmd

F32 = mybir.dt.float32
F32R = mybir.dt.float32r
EXP = mybir.ActivationFunctionType.Exp
IDENT = mybir.ActivationFunctionType.Identity

B, T, C = 8, 1024, 512
H = 8
D = C // H          # 64
NF = 128            # frames
NA = 8              # animals per frame
NT = T // 128       # 8 query/key tiles of 128
NC4 = C // 128      # 4 feature tiles
NEG = -1e9


def build_attention(tc, out_ap, ins, general_mask):
    """Emit the per-core attention program into TileContext tc.

    ins: dict of input APs (DRAM).
    """
    nc = tc.nc
    xT, wqT, wkT, wvT, wpT = ins["xT"], ins["wqT"], ins["wkT"], ins["wvT"], ins["wpT"]
    bq_t, bk_t = ins["bq_t"], ins["bk_t"]
    bv_row, bp_row = ins["bv_row"], ins["bp_row"]
    ones_in = ins["ones_in"]
    kaug_in, qaug_in = ins["kaug"], ins["qaug"]
    kmask = ins.get("kmask")

    # ---------------- persistent SBUF tiles ----------------
    frees = []

    def t_sb(name, shape, dtype=F32R):
        tl, free = tc.tile(shape, dtype, name=name)
        frees.append(free)
        return tl

    xt = [t_sb(f"xt{i}", [128, T]) for i in range(NC4)]
    wq = [t_sb(f"wq{i}", [128, C]) for i in range(NC4)]
    wk = [t_sb(f"wk{i}", [128, C]) for i in range(NC4)]
    wv = [t_sb(f"wv{i}", [128, C]) for i in range(NC4)]
    wp = [t_sb(f"wp{i}", [128, C]) for i in range(NC4)]
    qT = [t_sb(f"qT{i}", [128, T]) for i in range(NC4)]
    kT = [t_sb(f"kT{i}", [128, T]) for i in range(NC4)]
    # v per t-tile: [128, head, 66]; v cols 0:64, ones cols 64:66 (even
    # stationary free dim for fp32r; row 64 of the AV output = softmax denom)
    vt = [t_sb(f"vt{i}", [128, H, 66]) for i in range(NT)]
    yT = [t_sb(f"yT{i}", [128, T]) for i in range(NC4)]
    ones_sb = t_sb("ones_sb", [128, 128])
    kaug_sb = t_sb("kaug_sb", [17, 128])
    qaug_sb = t_sb("qaug_sb", [17, 512])
    bq_sb = t_sb("bq_sb", [128, NC4], F32)
    bk_sb = t_sb("bk_sb", [128, NC4], F32)
    bv_sb = t_sb("bv_sb", [1, C])
    bp_sb = t_sb("bp_sb", [1, C])
    km_sb = t_sb("km_sb", [128, NT], F32) if general_mask else None

    # ---------------- input DMAs ----------------
    for i in range(NC4):
        nc.sync.dma_start(out=xt[i][:, 0:512],
                          in_=xT[i * 128:(i + 1) * 128, 0:512].bitcast(F32R))
    for i in range(NC4):
        nc.sync.dma_start(out=wv[i], in_=wvT[i * 128:(i + 1) * 128, :].bitcast(F32R))
    for i in range(NC4):
        nc.sync.dma_start(out=xt[i][:, 512:T],
                          in_=xT[i * 128:(i + 1) * 128, 512:T].bitcast(F32R))
    for i in range(NC4):
        nc.sync.dma_start(out=wq[i], in_=wqT[i * 128:(i + 1) * 128, :].bitcast(F32R))
        nc.sync.dma_start(out=wk[i], in_=wkT[i * 128:(i + 1) * 128, :].bitcast(F32R))
    nc.sync.dma_start(out=kaug_sb, in_=kaug_in.bitcast(F32R))
    nc.sync.dma_start(out=qaug_sb, in_=qaug_in.bitcast(F32R))
    nc.sync.dma_start(out=bq_sb, in_=bq_t)
    nc.sync.dma_start(out=bk_sb, in_=bk_t)
    nc.sync.dma_start(out=bv_sb, in_=bv_row.bitcast(F32R))
    nc.sync.dma_start(out=bp_sb, in_=bp_row.bitcast(F32R))
    for i in range(NC4):
        nc.sync.dma_start(out=wp[i], in_=wpT[i * 128:(i + 1) * 128, :].bitcast(F32R))
    if general_mask:
        nc.sync.dma_start(out=km_sb, in_=kmask)
    nc.sync.dma_start(out=ones_sb, in_=ones_in.bitcast(F32R))
    for i in range(NT):
        # ones cols (softmax denominator rows in the AV matmul)
        nc.sync.dma_start(out=vt[i][:, :, 64:66],
                          in_=ones_in[:, 0:16].rearrange("p (h o) -> p h o", h=H).bitcast(F32R))

    # ---------------- pools ----------------
    ctx = contextlib.ExitStack()
    with ctx:
        mm_pool = ctx.enter_context(tc.tile_pool(name="mm", bufs=4, space="PSUM"))
        py_pool = ctx.enter_context(tc.tile_pool(name="py", bufs=2, space="PSUM"))
        pe_pool = ctx.enter_context(tc.tile_pool(name="pe", bufs=12))
        rr_pool = ctx.enter_context(tc.tile_pool(name="rr", bufs=2))
        ob_pool = ctx.enter_context(tc.tile_pool(name="ob", bufs=4))

        # ---------------- phase 1+2 interleaved ----------------
        def emit_v():
            for tt in range(NT):
                psv = py_pool.tile([128, T], F32, tag="py", name=f"psv{tt}")[:, 0:512]
                for c in range(NC4):
                    nc.tensor.matmul(
                        psv,
                        xt[c][:, tt * 128:(tt + 1) * 128],
                        wv[c],
                        start=(c == 0), stop=False)
                nc.tensor.matmul(psv, ones_sb[0:1, 0:128],
                                 bv_sb, start=False, stop=True)
                psv3 = psv.rearrange("p (h d) -> p h d", h=H)
                nc.vector.tensor_copy(vt[tt][:, :, 0:64], psv3)


        def emit_qk(i):
            if True:
                for ch in range(2):
                    tsl = slice(ch * 512, ch * 512 + 512)
                    psq = mm_pool.tile([128, 512], F32, tag="mm", name=f"psq{i}{ch}")
                    for c in range(NC4):
                        nc.tensor.matmul(
                            psq,
                            wq[c][:, i * 128:(i + 1) * 128],
                            xt[c][:, tsl],
                            start=(c == 0), stop=(c == NC4 - 1))
                    nc.vector.tensor_scalar_add(qT[i][:, tsl], psq,
                                                bq_sb[:, i:i + 1])
                    psk = mm_pool.tile([128, 512], F32, tag="mm", name=f"psk{i}{ch}")
                    for c in range(NC4):
                        nc.tensor.matmul(
                            psk,
                            wk[c][:, i * 128:(i + 1) * 128],
                            xt[c][:, tsl],
                            start=(c == 0), stop=(c == NC4 - 1))
                    nc.vector.tensor_scalar_add(kT[i][:, tsl], psk, bk_sb[:, i:i + 1])


        def emit_head(h):
            ht, hr = h // 2, (h % 2) * 64
            lrow = 64
            ysl = slice(0, 64)
            avsl = slice(0, 66)
            scale = 1.0 / math.sqrt(D)
            psY = py_pool.tile([128, T], F32, tag="py", name=f"psY{h}")
            for p in range(4):
                cols = slice(p * 256, p * 256 + 256)
                for kk in [p] + list(range(p)):
                    diag = kk == p
                    psS = mm_pool.tile([128, 512], F32, tag="mm",
                                       name=f"psS{h}{p}{kk}")
                    for half in range(2):
                        ki = 2 * kk + half
                        hsl = slice(half * 256, half * 256 + 256)
                        if diag:  # block-causal mask bias (0 / -BIG) first:
                            # depends only on constant tiles, off the
                            # score->exp critical path
                            nc.tensor.matmul(psS[:, hsl], kaug_sb,
                                             qaug_sb[:, hsl],
                                             start=True, stop=False)
                        nc.tensor.matmul(
                            psS[:, hsl],
                            kT[ht][hr:hr + 64, ki * 128:(ki + 1) * 128],
                            qT[ht][hr:hr + 64, cols],
                            start=not diag, stop=True)
                    pexp = pe_pool.tile([128, 512], F32R, tag="pe",
                                        name=f"pexp{h}{p}{kk}")
                    if general_mask:
                        for half in range(2):
                            ki = 2 * kk + half
                            hsl = slice(half * 256, half * 256 + 256)
                            nc.scalar.activation(
                                out=pexp[:, hsl], in_=psS[:, hsl], func=EXP,
                                bias=km_sb[:, ki:ki + 1], scale=scale)
                    else:
                        nc.scalar.activation(out=pexp, in_=psS, func=EXP,
                                             scale=scale)
                    first_ki = 2 * p
                    last_ki = 2 * p - 1 if p > 0 else 2 * p + 1
                    for half in range(2):
                        ki = 2 * kk + half
                        nc.tensor.matmul(
                            psY[avsl, cols],
                            vt[ki][:, h, :],
                            pexp[:, half * 256:half * 256 + 256],
                            start=(ki == first_ki), stop=(ki == last_ki))
            # normalize: r = 1/l, broadcast over 64 partitions, multiply
            # (two 512-col halves so the output projection can start on the
            # first half while the second is still in flight)
            rrow = rr_pool.tile([1, T], F32, tag="rr", name=f"rrow{h}")
            rrep = rr_pool.tile([64, T], F32, tag="rrep", name=f"rrep{h}")
            for cc in range(2):
                csl = slice(cc * 512, cc * 512 + 512)
                nc.vector.reciprocal(rrow[:, csl], psY[lrow:lrow + 1, csl])
                nc.gpsimd.partition_broadcast(rrep[:, csl], rrow[:, csl])
                nc.vector.tensor_mul(yT[ht][hr:hr + 64, csl], psY[ysl, csl],
                                     rrep[:, csl])


        emit_v()
        for i in range(NC4):
            emit_qk(i)
        for h in range(H):
            emit_head(h)

        # ---------------- phase 3: output projection ----------------
        for tt in range(NT):
            pso = mm_pool.tile([128, 512], F32, tag="mm", name=f"pso{tt}")
            for c in range(NC4):
                nc.tensor.matmul(
                    pso,
                    yT[c][:, tt * 128:(tt + 1) * 128],
                    wp[c],
                    start=(c == 0), stop=False)
            nc.tensor.matmul(pso, ones_sb[0:1, 0:128],
                             bp_sb, start=False, stop=True)
            o_sb = ob_pool.tile([128, 512], F32, tag="ob", name=f"osb{tt}")
            nc.scalar.copy(o_sb, pso)
            nc.sync.dma_start(out=out_ap[tt * 128:(tt + 1) * 128, :], in_=o_sb)

    for f in reversed(frees):
        f()


# ---------------------------------------------------------------------------
# host side
# ---------------------------------------------------------------------------

BIG = 8192.0


def _aug_mask_tiles():
    """Rank-17 additive encoding of the diagonal block-causal mask.

    bias[tk, c] = BIG * (w[tk//8, c] - 1): 0 where allowed, -BIG where masked.
    kaug [17, 128]: rows f<16: BIG * [tk//8 == f]; row 16: ones.
    qaug [17, 512]: cols 0:256 for key tile ki=2p (fine | allowed),
                    cols 256:512 for ki=2p+1 (masked | fine); row 16: -BIG."""
    a = np.arange(128)
    f = np.arange(16)
    kaug = np.zeros((17, 128), np.float32)
    kaug[:16] = BIG * (a[None, :] // NA == f[:, None])
    kaug[16] = 1.0
    fine = (a[None, :] // NA >= f[:, None]).astype(np.float32)  # [16, 128]
    qaug = np.zeros((17, 512), np.float32)
    qaug[:16, 0:128] = fine
    qaug[:16, 128:256] = 1.0
    qaug[:16, 256:384] = 0.0
    qaug[:16, 384:512] = fine
    qaug[16] = -BIG
    return kaug, qaug


def make_host_inputs(x, mask, Wq, bq, Wk, bk, Wv, bv, Wp, bp):
    """Returns (per_core_inputs, general_mask)."""
    f32 = np.float32
    x = np.asarray(x, dtype=f32)
    mask = np.asarray(mask, dtype=f32)
    Wq, bq = np.asarray(Wq, dtype=f32), np.asarray(bq, dtype=f32)
    Wk, bk = np.asarray(Wk, dtype=f32), np.asarray(bk, dtype=f32)
    Wv, bv = np.asarray(Wv, dtype=f32), np.asarray(bv, dtype=f32)
    Wp, bp = np.asarray(Wp, dtype=f32), np.asarray(bp, dtype=f32)
    general_mask = not bool(np.all(mask == 1.0))
    shared = {
        "wqT": np.ascontiguousarray(Wq.T.astype(f32)),
        "wkT": np.ascontiguousarray(Wk.T.astype(f32)),
        "wvT": np.ascontiguousarray(Wv.T.astype(f32)),
        "wpT": np.ascontiguousarray(Wp.T.astype(f32)),
        "bq_t": np.ascontiguousarray(bq.astype(f32).reshape(NC4, 128).T),
        "bk_t": np.ascontiguousarray(bk.astype(f32).reshape(NC4, 128).T),
        "bv_row": bv.astype(f32).reshape(1, C).copy(),
        "bp_row": bp.astype(f32).reshape(1, C).copy(),
        "ones_in": np.ones((128, 128), np.float32),
    }
    shared["kaug"], shared["qaug"] = _aug_mask_tiles()
    per_core = []
    for b in range(B):
        d = dict(shared)
        d["xT"] = np.ascontiguousarray(x[b].astype(f32).T)
        if general_mask:
            km = np.where(mask[b] != 0, 0.0, NEG).astype(f32)
            d["kmask"] = np.ascontiguousarray(km.reshape(NT, 128).T)
        per_core.append(d)
    return per_core, general_mask


def build_program(general_mask=False):
    nc = bacc.Bacc("TRN2", target_bir_lowering=False, debug=False, num_devices=1)
    ins = {
        "xT": nc.dram_tensor("xT", [C, T], F32, kind="ExternalInput").ap(),
        "wqT": nc.dram_tensor("wqT", [C, C], F32, kind="ExternalInput").ap(),
        "wkT": nc.dram_tensor("wkT", [C, C], F32, kind="ExternalInput").ap(),
        "wvT": nc.dram_tensor("wvT", [C, C], F32, kind="ExternalInput").ap(),
        "wpT": nc.dram_tensor("wpT", [C, C], F32, kind="ExternalInput").ap(),
        "bq_t": nc.dram_tensor("bq_t", [128, NC4], F32, kind="ExternalInput").ap(),
        "bk_t": nc.dram_tensor("bk_t", [128, NC4], F32, kind="ExternalInput").ap(),
        "bv_row": nc.dram_tensor("bv_row", [1, C], F32, kind="ExternalInput").ap(),
        "bp_row": nc.dram_tensor("bp_row", [1, C], F32, kind="ExternalInput").ap(),
        "kaug": nc.dram_tensor("kaug", [17, 128], F32, kind="ExternalInput").ap(),
        "qaug": nc.dram_tensor("qaug", [17, 512], F32, kind="ExternalInput").ap(),
        "ones_in": nc.dram_tensor("ones_in", [128, 128], F32,
                                  kind="ExternalInput").ap(),
    }
    if general_mask:
        ins["kmask"] = nc.dram_tensor("kmask", [128, NT], F32,
                                      kind="ExternalInput").ap()
    out = nc.dram_tensor("out", [T, C], F32, kind="ExternalOutput").ap()
    with tile.TileContext(nc) as tc:
        build_attention(tc, out, ins, general_mask)
    nc.compile()
    return nc


_cached = {}


def get_program(general_mask=False):
    if general_mask not in _cached:
        _cached[general_mask] = build_program(general_mask)
    return _cached[general_mask]


def kernel(x, mask, Wq, bq, Wk, bk, Wv, bv, Wp, bp):
    per_core, general_mask = make_host_inputs(
        x, mask, Wq, bq, Wk, bk, Wv, bv, Wp, bp)
    nc = get_program(general_mask)
    res = run_bass_kernel_spmd(nc, per_core, core_ids=list(range(B)))
    out = np.stack([res.results[b]["out"] for b in range(B)], axis=0)
    return out.astype(np.float32)



# revision 19
# speedup vs baseline: 1.2284x; 1.2284x over previous
"""Block-causal self-attention on 8 Trainium2 NeuronCores.

Sharding: data-parallel over batch (B=8 -> one batch element per core).
Weights replicated. No collectives.

Per-core Bass program (fp16 storage, fp32 PSUM accumulation):
  - inputs arrive pre-transposed on host in fp16: xT=[C,T], w*T=[C,C],
    batched into few large multi-dim DMAs (HWDGE serializes per-DMA)
  - qT,kT = W @ xT + b   (feature-on-partition layout [128, T] head pairs;
    bias added during the PSUM->SBUF fp16 conversion on DVE)
  - v stored as one tile [128, tile, head, 128] fp16: cols 0:64 v,
    cols 64:128 ones (the AV output rows 64:128 are then the softmax
    denominator replicated 64x -> normalize is a single tensor-tensor div)
  - per (head, key-tile j), transposed-scores flash attention over the
    query span q in [128j, 1024) only (block-causal: skip upper triangle):
      scores^T[key, query] = kT_tile.T @ qT  (K=d=64)
      diagonal 128x128 gets the fine mask as a rank-17 fp16 matmul
      ACT exp (scale fused; general mask adds per-key bias) -> pexp fp16
    AV per query-tile i: psY slot i [128,128] += vt[j].T @ pexp_j, j=0..i
  - normalize: yT = psY[0:64] / psY[64:128] (one tensor-tensor divide)
  - out = yT proj (fp16 matmuls, f32 psum); bias via tensor add; DMA out
  - emission interleaves QK GEMM units and the output projection into the
    attention stream so the PE never starves while ACT exponentiates
"""

import contextlib
import math

import numpy as np

import concourse.bass as bass
import concourse.mybir as mybir
import concourse.tile as tile
from concourse import bacc
from concourse.bass_utils import run_bass_kernel_spmd

F32 = mybir.dt.float32
F16 = mybir.dt.float16
EXP = mybir.ActivationFunctionType.Exp
DIV = mybir.AluOpType.divide
ADD = mybir.AluOpType.add

B, T, C = 8, 1024, 512
H = 8
D = C // H          # 64
NF = 128            # frames
NA = 8              # animals per frame
NT = T // 128       # 8 query/key tiles of 128
NC4 = C // 128      # 4 feature tiles
NEG = -1e9
BIG = 8192.0

# engine knobs (validated on sim+hw)
NORM_ENGINE = "dve"       # "pool" | "dve"  - the psY divide ("pool" is
                          # sim-legal but real GPSIMD cannot access PSUM)
PROJ_ADD_ENGINE = "dve"   # "pool" | "dve"  - the proj bias add


def build_attention(tc, out_ap, ins, general_mask):
    nc = tc.nc
    xT, wqT, wkT, wvT, wpT = ins["xT"], ins["wqT"], ins["wkT"], ins["wvT"], ins["wpT"]
    bq_t, bk_t = ins["bq_t"], ins["bk_t"]
    bv_rep, bp_rep = ins["bv_rep"], ins["bp_rep"]
    kaug_in, qaug_in = ins["kaug"], ins["qaug"]
    ones_in = ins["ones2"]
    kmask = ins.get("kmask")
    scale = 1.0 / math.sqrt(D)

    # ---------------- persistent SBUF tiles ----------------
    frees = []

    def t_sb(name, shape, dtype=F16, space="SBUF"):
        tl, free = tc.tile(shape, dtype, name=name, space=space)
        frees.append(free)
        return tl

    xt = t_sb("xt", [128, NC4, T])          # [p, c-chunk, t]
    wq = t_sb("wq", [128, NC4, C])
    wk = t_sb("wk", [128, NC4, C])
    wv = t_sb("wv", [128, NC4, C])
    wp = t_sb("wp", [128, NC4, C])
    qT = [t_sb(f"qT{i}", [128, T]) for i in range(NC4)]
    kT = [t_sb(f"kT{i}", [128, T]) for i in range(NC4)]
    # v: [128, key-tile, head, 128]; cols 0:64 v, cols 64:128 ones (the ones
    # columns make AV emit the softmax denominator replicated 64x)
    vt = t_sb("vt", [128, NT, H, 128])
    yT = [t_sb(f"yT{i}", [128, T]) for i in range(NC4)]
    kaug_sb = t_sb("kaug_sb", [17, 128])
    qaug_sb = t_sb("qaug_sb", [17, 128])
    bq_sb = t_sb("bq_sb", [128, NC4], F32)
    bk_sb = t_sb("bk_sb", [128, NC4], F32)
    bv_sb = t_sb("bv_sb", [128, C], F32)
    bp_sb = t_sb("bp_sb", [128, C], F32)
    km_sb = t_sb("km_sb", [128, NT], F32) if general_mask else None
    # AV accumulators: 8 slots (one per query tile) split across two psum
    # tensors (1 bank each) so slot i's accumulation never serializes
    # against slot i-1's drain
    psYs = [t_sb(f"psY{k}", [128, NT // 2, 128], F32, space="PSUM")
            for k in range(2)]

    # ---------------- input DMAs (few, large, need-ordered) ----------------
    def w_src(wT):
        return wT.rearrange("(c p) n -> p c n", p=128)

    nc.sync.dma_start(out=xt[:, 0:2, 0:512],
                      in_=xT.rearrange("(c p) t -> p c t", p=128)[:, 0:2, 0:512])
    nc.sync.dma_start(out=wv[:, 0:2, :], in_=w_src(wvT)[:, 0:2, :])
    nc.sync.dma_start(out=xt[:, 2:4, 0:512],
                      in_=xT.rearrange("(c p) t -> p c t", p=128)[:, 2:4, 0:512])
    nc.sync.dma_start(out=wv[:, 2:4, :], in_=w_src(wvT)[:, 2:4, :])
    nc.sync.dma_start(out=bv_sb, in_=bv_rep)
    nc.sync.dma_start(out=wq, in_=w_src(wqT))
    nc.sync.dma_start(out=wk, in_=w_src(wkT))
    nc.sync.dma_start(out=xt[:, :, 512:T],
                      in_=xT.rearrange("(c p) t -> p c t", p=128)[:, :, 512:T])
    nc.sync.dma_start(out=bq_sb, in_=bq_t)
    nc.sync.dma_start(out=bk_sb, in_=bk_t)
    nc.sync.dma_start(out=kaug_sb, in_=kaug_in)
    nc.sync.dma_start(out=qaug_sb, in_=qaug_in)
    nc.sync.dma_start(out=vt.rearrange("p j h o -> p (j h) o")[:, :, 64:128],
                      in_=ones_in)
    nc.sync.dma_start(out=bp_sb, in_=bp_rep)
    nc.sync.dma_start(out=wp, in_=w_src(wpT))
    if general_mask:
        nc.sync.dma_start(out=km_sb, in_=kmask)

    # ---------------- pools ----------------
    ctx = contextlib.ExitStack()
    with ctx:
        mm_pool = ctx.enter_context(tc.tile_pool(name="mm", bufs=2, space="PSUM"))
        ps_pool = ctx.enter_context(tc.tile_pool(name="ps", bufs=4, space="PSUM"))
        pe_pool = ctx.enter_context(tc.tile_pool(name="pe", bufs=16))
        ob_pool = ctx.enter_context(tc.tile_pool(name="ob", bufs=3))
        rr_pool = ctx.enter_context(tc.tile_pool(name="rr", bufs=4))

        def emit_v_pair(tp):
            # two V tiles interleaved at half-contraction granularity so the
            # first matmuls only need the first xt/wv half-DMAs
            psvs = [mm_pool.tile([128, 512], F32, tag="mm", name=f"psv{tt}")
                    for tt in (2 * tp, 2 * tp + 1)]
            for c2 in range(2):
                for k, tt in enumerate((2 * tp, 2 * tp + 1)):
                    for c in (2 * c2, 2 * c2 + 1):
                        nc.tensor.matmul(
                            psvs[k], xt[:, c, tt * 128:(tt + 1) * 128],
                            wv[:, c, :],
                            start=(c == 0), stop=(c == NC4 - 1))
            for k, tt in enumerate((2 * tp, 2 * tp + 1)):
                nc.vector.tensor_add(
                    vt[:, tt, :, 0:64],
                    psvs[k].rearrange("p (h d) -> p h d", h=H),
                    bv_sb.rearrange("p (h d) -> p h d", h=H))

        def emit_qk_unit(p, ch, is_k):
            tsl = slice(ch * 512, ch * 512 + 512)
            w, dst, b_sb = (wk, kT, bk_sb) if is_k else (wq, qT, bq_sb)
            psx = mm_pool.tile([128, 512], F32, tag="mm",
                               name=f"ps{'k' if is_k else 'q'}{p}{ch}")
            for c in range(NC4):
                nc.tensor.matmul(
                    psx, w[:, c, p * 128:(p + 1) * 128], xt[:, c, tsl],
                    start=(c == 0), stop=(c == NC4 - 1))
            nc.vector.tensor_scalar_add(dst[p][:, tsl], psx, b_sb[:, p:p + 1])

        def emit_proj(tt):
            pso = mm_pool.tile([128, 512], F32, tag="mm", name=f"pso{tt}")
            for c in range(NC4):
                nc.tensor.matmul(
                    pso, yT[c][:, tt * 128:(tt + 1) * 128], wp[:, c, :],
                    start=(c == 0), stop=(c == NC4 - 1))
            o_sb = ob_pool.tile([128, 512], F32, tag="ob", name=f"osb{tt}")
            eng = nc.gpsimd if PROJ_ADD_ENGINE == "pool" else nc.vector
            eng.tensor_tensor(out=o_sb, in0=pso, in1=bp_sb, op=ADD)
            nc.sync.dma_start(out=out_ap[tt * 128:(tt + 1) * 128, :], in_=o_sb)

        def emit_head(h, fillers):
            ht, hr = h // 2, (h % 2) * 64
            pexps = {}

            def scores_chunk(j, c):
                # chunk c of key-tile j's query span [128j, 1024): chunk 0 is
                # span-relative cols [0:512) (diag + off-diag), chunk 1 is
                # [512:span). Each chunk is one psum bank.
                span0 = j * 128
                span = T - span0
                lo, hi = (0, min(512, span)) if c == 0 else (512, span)
                psS = ps_pool.tile([128, 512], F32, tag="ps",
                                   name=f"psS{h}_{j}_{c}")
                if c == 0:
                    # diagonal 128x128: rank-17 fine-mask bias first
                    # (constants, off the critical path), then the scores
                    nc.tensor.matmul(psS[:, 0:128], kaug_sb, qaug_sb,
                                     start=True, stop=False)
                    nc.tensor.matmul(
                        psS[:, 0:128],
                        kT[ht][hr:hr + 64, span0:span0 + 128],
                        qT[ht][hr:hr + 64, span0:span0 + 128],
                        start=False, stop=True)
                    if hi > 128:
                        nc.tensor.matmul(
                            psS[:, 128:hi],
                            kT[ht][hr:hr + 64, span0:span0 + 128],
                            qT[ht][hr:hr + 64, span0 + 128:span0 + hi],
                            start=True, stop=True)
                else:
                    nc.tensor.matmul(
                        psS[:, 0:hi - lo],
                        kT[ht][hr:hr + 64, span0:span0 + 128],
                        qT[ht][hr:hr + 64, span0 + lo:span0 + hi],
                        start=True, stop=True)
                pexp = pe_pool.tile([128, 512], F16, tag="pe",
                                    name=f"pexp{h}_{j}_{c}")
                bias = km_sb[:, j:j + 1] if general_mask else 0.0
                nc.scalar.activation(out=pexp[:, 0:hi - lo],
                                     in_=psS[:, 0:hi - lo],
                                     func=EXP, bias=bias, scale=scale)
                pexps[(j, c)] = pexp

            def av(i):
                slot = psYs[i % 2][:, i // 2, :]
                for j in range(i + 1):
                    rel = (i - j) * 128
                    c, off = (1, rel - 512) if rel >= 512 else (0, rel)
                    nc.tensor.matmul(slot, vt[:, j, h, :],
                                     pexps[(j, c)][:, off:off + 128],
                                     start=(j == 0), stop=(j == i))
                # hw allows only one PSUM input per instruction: reciprocal
                # of the replicated denominator rows to SBUF, then multiply
                rr = rr_pool.tile([64, 128], F32, tag="rr", name=f"rr{h}{i}")
                nc.vector.reciprocal(rr, slot[64:128, :])
                nc.vector.tensor_mul(
                    yT[ht][hr:hr + 64, i * 128:(i + 1) * 128],
                    slot[0:64, :], rr)

            C0 = lambda j: ("c", j, 0)
            C1 = lambda j: ("c", j, 1)
            AV = lambda i: ("a", i)
            FILL = ("f",)
            seq = [C0(0), C0(1), C0(2), AV(0), C0(3), FILL, AV(1),
                   C1(0), AV(2), C0(4), FILL, AV(3), C1(1), AV(4),
                   C0(5), FILL, C1(2), AV(5), C0(6), FILL, C1(3), AV(6),
                   C0(7), FILL, FILL, AV(7)]
            for step in seq:
                if step[0] == "c":
                    scores_chunk(step[1], step[2])
                elif step[0] == "a":
                    av(step[1])
                elif fillers:
                    fillers.pop(0)()
            while fillers:
                fillers.pop(0)()

        for tp in range(NT // 2):
            emit_v_pair(tp)
        for ch in range(2):
            emit_qk_unit(0, ch, False)
            emit_qk_unit(0, ch, True)
        # QK units for pairs 1..3 and the projection tiles become fillers
        # inside earlier heads: pair p's units run inside heads < 2p, proj
        # tile tt inside head 7 after its av(tt).
        filler_assign = {hh: [] for hh in range(H)}
        units = [(p, ch, is_k) for p in (1, 2, 3) for ch in range(2)
                 for is_k in (False, True)]
        head_for = [0, 0, 1, 1, 2, 2, 3, 3, 4, 4, 5, 5]
        for (p, ch, is_k), hh in zip(units, head_for):
            filler_assign[hh].append(
                lambda p=p, ch=ch, is_k=is_k: emit_qk_unit(p, ch, is_k))
        for tt in range(6):
            filler_assign[7].append(lambda tt=tt: emit_proj(tt))
        for h in range(H):
            emit_head(h, filler_assign[h])
        for tt in range(6, NT):
            emit_proj(tt)

    for f in reversed(frees):
        f()


# ---------------------------------------------------------------------------
# host side
# ---------------------------------------------------------------------------


def _aug_mask_tiles():
    """Rank-17 additive encoding of the diagonal block-causal fine mask.

    bias[tk, tq] = BIG * (fine[tk, tq] - 1): 0 where allowed, -BIG where
    masked.  kaug [17, 128]: rows f<16: BIG * [tk//8 == f]; row 16: ones.
    qaug [17, 128]: rows f<16: [tq//8 >= f]; row 16: -BIG."""
    a = np.arange(128)
    f = np.arange(16)
    kaug = np.zeros((17, 128), np.float16)
    kaug[:16] = (BIG * (a[None, :] // NA == f[:, None])).astype(np.float16)
    kaug[16] = 1.0
    qaug = np.zeros((17, 128), np.float16)
    qaug[:16] = (a[None, :] // NA >= f[:, None]).astype(np.float16)
    qaug[16] = -BIG
    return kaug, qaug


def make_host_inputs(x, mask, Wq, bq, Wk, bk, Wv, bv, Wp, bp):
    """Returns (per_core_inputs, general_mask)."""
    f32, f16 = np.float32, np.float16
    x = np.asarray(x, dtype=f32)
    mask = np.asarray(mask, dtype=f32)
    Wq, bq = np.asarray(Wq, dtype=f32), np.asarray(bq, dtype=f32)
    Wk, bk = np.asarray(Wk, dtype=f32), np.asarray(bk, dtype=f32)
    Wv, bv = np.asarray(Wv, dtype=f32), np.asarray(bv, dtype=f32)
    Wp, bp = np.asarray(Wp, dtype=f32), np.asarray(bp, dtype=f32)
    general_mask = not bool(np.all(mask == 1.0))
    shared = {
        "wqT": np.ascontiguousarray(Wq.T.astype(f16)),
        "wkT": np.ascontiguousarray(Wk.T.astype(f16)),
        "wvT": np.ascontiguousarray(Wv.T.astype(f16)),
        "wpT": np.ascontiguousarray(Wp.T.astype(f16)),
        "bq_t": np.ascontiguousarray(bq.reshape(NC4, 128).T).astype(f32),
        "bk_t": np.ascontiguousarray(bk.reshape(NC4, 128).T).astype(f32),
        "bv_rep": np.tile(bv.reshape(1, C), (128, 1)).astype(f32),
        "bp_rep": np.tile(bp.reshape(1, C), (128, 1)).astype(f32),
        "ones2": np.ones((128, NT * H * 64), f16),
    }
    shared["kaug"], shared["qaug"] = _aug_mask_tiles()
    per_core = []
    for b in range(B):
        d = dict(shared)
        d["xT"] = np.ascontiguousarray(x[b].T).astype(f16)
        if general_mask:
            km = np.where(mask[b] != 0, 0.0, NEG).astype(f32)
            d["kmask"] = np.ascontiguousarray(km.reshape(NT, 128).T)
        per_core.append(d)
    return per_core, general_mask


def build_program(general_mask=False):
    nc = bacc.Bacc("TRN2", target_bir_lowering=False, debug=False, num_devices=1)
    ins = {
        "xT": nc.dram_tensor("xT", [C, T], F16, kind="ExternalInput").ap(),
        "wqT": nc.dram_tensor("wqT", [C, C], F16, kind="ExternalInput").ap(),
        "wkT": nc.dram_tensor("wkT", [C, C], F16, kind="ExternalInput").ap(),
        "wvT": nc.dram_tensor("wvT", [C, C], F16, kind="ExternalInput").ap(),
        "wpT": nc.dram_tensor("wpT", [C, C], F16, kind="ExternalInput").ap(),
        "bq_t": nc.dram_tensor("bq_t", [128, NC4], F32, kind="ExternalInput").ap(),
        "bk_t": nc.dram_tensor("bk_t", [128, NC4], F32, kind="ExternalInput").ap(),
        "bv_rep": nc.dram_tensor("bv_rep", [128, C], F32, kind="ExternalInput").ap(),
        "bp_rep": nc.dram_tensor("bp_rep", [128, C], F32, kind="ExternalInput").ap(),
        "kaug": nc.dram_tensor("kaug", [17, 128], F16, kind="ExternalInput").ap(),
        "qaug": nc.dram_tensor("qaug", [17, 128], F16, kind="ExternalInput").ap(),
        "ones2": nc.dram_tensor("ones2", [128, NT * H * 64], F16,
                                kind="ExternalInput").ap(),
    }
    if general_mask:
        ins["kmask"] = nc.dram_tensor("kmask", [128, NT], F32,
                                      kind="ExternalInput").ap()
    out = nc.dram_tensor("out", [T, C], F32, kind="ExternalOutput").ap()
    with tile.TileContext(nc) as tc:
        build_attention(tc, out, ins, general_mask)
    nc.compile()
    return nc


_cached = {}


def get_program(general_mask=False):
    if general_mask not in _cached:
        _cached[general_mask] = build_program(general_mask)
    return _cached[general_mask]


def kernel(x, mask, Wq, bq, Wk, bk, Wv, bv, Wp, bp):
    per_core, general_mask = make_host_inputs(
        x, mask, Wq, bq, Wk, bk, Wv, bv, Wp, bp)
    nc = get_program(general_mask)
    res = run_bass_kernel_spmd(nc, per_core, core_ids=list(range(B)))
    out = np.stack([res.results[b]["out"] for b in range(B)], axis=0)
    return out.astype(np.float32)
